# revision 37
# baseline (speedup 1.0000x reference)
"""Trainium2 Bass kernel for multi-head attention (B=1, N=4096, C=768, H=12, D=64).

Sharding: tensor-parallel over heads across 8 cores. Core c (pair k=c//2):
  even c: head A = 3k   (all 8 query blocks), head B = 3k+1 (query blocks 0-3)
  odd  c: head A = 3k+2 (all 8 query blocks), head B = 3k+1 (query blocks 4-7)
The SPMD program is identical on every core; odd cores receive x^T with its
columns rotated by 2048 so that "local query blocks 0-3" of head B are the
global blocks 4-7.  The host un-permutes rows, normalizes by the softmax row
sums (computed on device via a ones-column appended to V), sums the per-core
partial projections and adds the bias.

v3 (cost-model driven):
  Q/K projection + scores + AV run as fp8e4 DoubleRow matmuls (2 k-subtiles
  per pass, 0.5 cyc/row).  Q^T/K^T live as [32, 2, N] fp8 tiles (d split in
  two 32-row k-tiles, W_qkv scaled x32; the x32*x32 factor is folded into the
  exp scale).  V is computed from bf16 x / bf16 W_v (fp8 V noise would pass
  straight to the output) and stored fp8 [128, 16, 2, 2, 65] with a ones
  column at dv=64 that yields softmax row sums in acc row 64.
  Scores S^T [m, q] accumulate in PSUM [128, 3, 512] tiles (3 key blocks per
  exp call); exp is split between ScalarE (exact, exp -> fp8) and VectorE
  (fast-exp: affine -> int8 -> bitcast fp8) with greedy load balancing,
  writing into a monolithic per-unit P^T tile [128, 32, 512] fp8 so AV can
  consume mb PAIRS via DoubleRow regardless of the exp grouping.
  Emission runs on an explicit step scheduler: scores(s) | AVs enabled by
  step s-1's exp | staged finalize (osb copy, then proj+y a step later);
  QKV projection blocks are interleaved right before the first score group
  that needs them.  Input DMAs split across the SP queue (fp8 x, qk weights)
  and the Pool/gpsimd queue (bf16 x, V/proj weights, all outputs).
"""

import sys

for _p in ("/opt/trn_rl_repo",):
    if _p not in sys.path:
        sys.path.insert(0, _p)

import ml_dtypes
import numpy as np

import concourse.bass as bass  # noqa: F401
import concourse.mybir as mybir
from concourse import bacc, tile
from concourse.bass_utils import run_bass_kernel_spmd

F32 = mybir.dt.float32
F32R = mybir.dt.float32r
BF16 = mybir.dt.bfloat16
FP8 = mybir.dt.float8e4
I8 = mybir.dt.int8
AF = mybir.ActivationFunctionType
DR = mybir.MatmulPerfMode.DoubleRow
MUL = mybir.AluOpType.mult
ADD = mybir.AluOpType.add
SUB = mybir.AluOpType.subtract

N = 4096
C = 768
D = 64
NB = 8  # 512-query/key blocks
WS = 32.0  # Q/K weight pre-scale (folded back out inside exp)
EXP_SCALE = (D ** -0.5) / (WS * WS)  # = 1/8192
# DVE fast-exp: i8 = round(raw * ALPHA + BETA); bitcast i8 -> fp8e4 ~ exp(raw*EXP_SCALE)
ALPHA = 8.0 * np.log2(np.e) * EXP_SCALE  # 1.4427/1024
BETA = 7 * 8 - 0.45  # fp8e4 bias 7 << 3 mantissa bits, Schraudolph-style centering

# groups of 2 key-blocks per exp call (PSUM: 2-bank score tiles x3 bufs)
GROUPS = [(2 * g, 2 * g + 2) for g in range(16)]
NG = len(GROUPS)
GW = 2  # max key-blocks per group

# per-free-element engine cost (ns) + fixed per-instruction cost, for greedy balance
ACT_CYC, ACT_FIX = 1.0 / 1.2, 242.0
DVE_CYC, DVE_FIX = 1.0 / 0.96, 230.0

_NC = None


def _emit(nc, tc, io, ctx):
    xb_in, w8_in, wvb_in, wp_in, y_out, rs_out = (
        io["xb"], io["w8"], io["wvb"], io["wp"], io["y"], io["rs"])

    sing = ctx.enter_context(tc.tile_pool(name="sing", bufs=1))
    spsum = ctx.enter_context(tc.tile_pool(name="spsum", bufs=3, space="PSUM"))
    apsum = ctx.enter_context(tc.tile_pool(name="apsum", bufs=2, space="PSUM"))
    ptp = ctx.enter_context(tc.tile_pool(name="ptp", bufs=3))
    osbp = ctx.enter_context(tc.tile_pool(name="osbp", bufs=3))
    ysbp = ctx.enter_context(tc.tile_pool(name="ysbp", bufs=6))

    # ---- greedy ACT/DVE load balancing for elementwise PSUM->SBUF work ----
    ew_load = {"act": 0.0, "dve": 0.0}

    def ew_pick(free):
        ca = ew_load["act"] + free * ACT_CYC + ACT_FIX
        cd = ew_load["dve"] + free * DVE_CYC + DVE_FIX
        if ca <= cd:
            ew_load["act"] = ca
            return "act"
        ew_load["dve"] = cd
        return "dve"

    def ew_copy(dst, src, free):
        if ew_pick(free) == "act":
            nc.scalar.copy(dst, src)
        else:
            nc.vector.tensor_copy(dst, src)

    def ew_exp(pt, ps, free):
        import os
        if os.environ.get("ALL_ACT_EXP") or ew_pick(free) == "act":
            nc.scalar.activation(out=pt, in_=ps, func=AF.Exp, scale=EXP_SCALE)
        else:
            nc.vector.tensor_scalar(pt.bitcast(I8), ps, ALPHA, BETA, MUL, ADD)

    # ---- weights: qk packed [K*32 | Q*32] bf16 (scores need bf16-grade
    # projection compute; only the Q/K stores are fp8 for the DR contract) ----
    wqk = {}
    for s, name in ((0, "wqk_a"), (1, "wqk_b")):
        t = sing.tile([128, 6, 128], BF16, name=name, tag=name)
        nc.sync.dma_start(
            out=t, in_=w8_in[name].rearrange("(cc p) d -> p cc d", p=128))
        wqk[s] = t
    wvb = sing.tile([128, 6, 128], BF16, name="wvb", tag="wvb")
    wp = {0: sing.tile([64, C], F32R, name="wp_a", tag="wp_a"),
          1: sing.tile([64, C], F32R, name="wp_b", tag="wp_b")}

    # ---- projection result tiles ----
    KT8 = [sing.tile([32, 2, N], FP8, name="kt_a", tag="kt_a"),
           sing.tile([32, 2, N], FP8, name="kt_b", tag="kt_b")]
    QT8 = [sing.tile([32, 2, N], FP8, name="qt_a", tag="qt_a"),
           sing.tile([32, 2, N // 2], FP8, name="qt_b", tag="qt_b")]
    # last dim padded 65->80 so the AV DoubleRow k-tile stride (160) is 16-aligned.
    # V is stored as fp8 value + fp8 residual (V ~ V8 + R8) because V-element
    # quantization noise passes straight through to the output.
    V = sing.tile([128, 16, 2, 2, 80], FP8, name="v", tag="v")
    nc.vector.memset(V[:, :, :, :, 64:65], 1.0)
    R = sing.tile([128, 16, 2, 2, 80], FP8, name="vr", tag="vr")
    nc.vector.memset(R[:, :, :, :, 64:65], 0.0)

    # ---- x tiles: host pre-arranged per-nb contiguous slabs, nb-major so
    # the qkv pipeline can start early ----
    xbt = [sing.tile([128, 6, 512], BF16, name=f"xb_{nb}", tag=f"xb_{nb}")
           for nb in range(NB)]
    for nb in range(NB):
        nc.sync.dma_start(out=xbt[nb], in_=xb_in[nb])
        if nb == 0:
            nc.sync.dma_start(
                out=wvb, in_=wvb_in.rearrange("(cc p) d -> p cc d", p=128))
        elif nb == 1:
            nc.sync.dma_start(out=wp[0], in_=wp_in[0])
            nc.sync.dma_start(out=wp[1], in_=wp_in[1])

    # ---- QKV projections for one 512-column block ----
    # bf16 matmuls compute K AND Q for a head: psum partitions 0:64 K^T d,
    # 64:128 Q^T d; partition-shifted fp8 copies split it into the
    # [32, 2, N] DoubleRow score layout.
    def emit_qk_proj(s, nb):
        pq = spsum.tile([128, 512], F32, name="pq", tag="big")
        for cc in range(6):
            nc.tensor.matmul(pq, lhsT=wqk[s][:, cc, :], rhs=xbt[nb][:, cc, :],
                             start=(cc == 0), stop=(cc == 5))
        sl = slice(nb * 512, (nb + 1) * 512)
        ew_copy(KT8[s][:, 0, sl], pq[0:32, :], 512)
        ew_copy(KT8[s][:, 1, sl], pq[32:64, :], 512)
        if s == 0 or nb < 4:
            ew_copy(QT8[s][:, 0, sl], pq[64:96, :], 512)
            ew_copy(QT8[s][:, 1, sl], pq[96:128, :], 512)

    def emit_v_proj(nb):
        for idx in range(4):
            mb = nb * 4 + idx
            psv = spsum.tile([128, 2, 64], F32, name="psv", tag="big")
            for cc in range(6):
                nc.tensor.matmul(psv,
                                 lhsT=xbt[nb][:, cc, idx * 128:(idx + 1) * 128],
                                 rhs=wvb[:, cc, :],
                                 start=(cc == 0), stop=(cc == 5))
            g, j = mb // 2, mb % 2
            ew_copy(V[:, g, j, :, 0:64], psv, 128)
            # fp8 residual on DVE: R8 = psv - float(V8)
            ew_load["dve"] += 128 * DVE_CYC + DVE_FIX
            nc.vector.scalar_tensor_tensor(R[:, g, j, :, 0:64], psv, 1.0,
                                           V[:, g, j, :, 0:64], MUL, SUB)

    emitted_nb = 0

    def need_nb(nb):
        nonlocal emitted_nb
        while emitted_nb <= nb:
            b = emitted_nb
            emit_qk_proj(0, b)
            emit_qk_proj(1, b)
            emit_v_proj(b)
            emitted_nb += 1

    # ---- attention: 6 pair-slots, units = (slot, local qb) ----
    pairs = [((0, 0), (1, 0)), ((0, 1), (1, 1)), ((0, 2), (1, 2)),
             ((0, 3), (1, 3)), ((0, 4), (0, 5)), ((0, 6), (0, 7))]

    acc = {}
    for ulo, uup in pairs:
        for u in (ulo, uup):
            acc[u] = apsum.tile([65, 512], F32, name=f"acc_{u[0]}_{u[1]}",
                                tag="acc")

    def av_ready(g):
        # mb pairs fully covered by exp groups 0..g
        return min(16, (GROUPS[g][1]) // 2) if g >= 0 else 0

    def emit_av(pair, pt, k0, k1):
        for k in range(k0, k1):
            for u in pair:
                s, qb = u
                rhs = pt[u][:, 2 * k:2 * k + 2, :]
                nc.tensor.matmul(acc[u], lhsT=V[:, k, :, s, 0:65], rhs=rhs,
                                 start=(k == 0), stop=False,
                                 perf_mode=DR, skip_group_check=True)
                nc.tensor.matmul(acc[u], lhsT=R[:, k, :, s, 0:65], rhs=rhs,
                                 start=False, stop=(k == 15),
                                 perf_mode=DR, skip_group_check=True)

    def emit_osb(pair, osb_box):
        for u in pair:
            s, qb = u
            o = osbp.tile([65, 512], F32R, name="osb", tag="osb")
            ew_copy(o, acc[u], 512)
            nc.sync.dma_start(out=rs_out[s][qb:qb + 1, :], in_=o[64:65, :])
            osb_box[u] = o

    def emit_proj(u, osb):
        s, qb = u
        for qs in range(4):
            py = spsum.tile([128, C], F32, name="py", tag="big")
            lw = osb[0:64, qs * 128:(qs + 1) * 128]
            nc.tensor.matmul(py[:, 0:512], lhsT=lw, rhs=wp[s][:, 0:512],
                             start=True, stop=True)
            nc.tensor.matmul(py[:, 512:C], lhsT=lw, rhs=wp[s][:, 512:C],
                             start=True, stop=True)
            ysb = ysbp.tile([128, C], F32, name="ysb", tag="ysb")
            ew_copy(ysb, py, C)
            row = qb * 512 + qs * 128
            nc.sync.dma_start(out=y_out[s][row:row + 128, :], in_=ysb)

    # ---- explicit step scheduler ----
    # step s: scores+exp for (pair_s, g_s); AVs enabled by step s-2's exp
    # (two-step lag hides exp latency from the in-order PE stream); after a
    # pair's last AV batch: osb copies one step later, then each unit's
    # projection staged on the following steps.
    steps = [(pair, g) for pair in pairs for g in range(NG)]
    pt_tiles = {}
    av_done = {}
    deferred = []  # (due_step, fn)

    def flush_deferred(s):
        nonlocal deferred
        deferred, due = ([d for d in deferred if d[0] > s],
                         [d for d in deferred if d[0] <= s])
        for _, fn in sorted(due, key=lambda d: d[0]):
            fn()

    for s, (pair, g) in enumerate(steps):
        m0, m1 = GROUPS[g]
        need_nb(max(pair[0][1], pair[1][1], (m1 - 1) // 4))
        if g == 0:
            pt_tiles[pair] = {
                u: ptp.tile([128, 32, 512], FP8, name=f"pt_{u[0]}_{u[1]}",
                            tag="pt")
                for u in pair}
            av_done[pair] = 0
        # scores for this group
        ps = {}
        for u in pair:
            ps[u] = spsum.tile([128, GW, 512], F32, name="ps_s", tag="big")
            sl, qb = u
            for j in range(m1 - m0):
                mb = m0 + j
                nc.tensor.matmul(
                    ps[u][:, j, :],
                    lhsT=KT8[sl][:, :, mb * 128:(mb + 1) * 128],
                    rhs=QT8[sl][:, :, qb * 512:(qb + 1) * 512],
                    start=True, stop=True, perf_mode=DR)
        # deferred AV batches / finalize stages due at this step
        flush_deferred(s)
        # exp for this group
        for u in pair:
            w = (m1 - m0) * 512
            ew_exp(pt_tiles[pair][u][:, m0:m1, :], ps[u][:, 0:m1 - m0, :], w)
        # schedule this group's AVs two steps out
        k1 = av_ready(g)

        def av_batch(p=pair, a0=av_done[pair], a1=k1):
            emit_av(p, pt_tiles[p], a0, a1)
        deferred.append((s + 2, av_batch))
        av_done[pair] = k1
        if g == NG - 1:
            box = {}

            def osb_batch(p=pair, bb=box):
                emit_osb(p, bb)
            deferred.append((s + 3, osb_batch))
            for i, u in enumerate(pair):
                deferred.append((s + 4 + i, (lambda uu=u, bb=box:
                                             emit_proj(uu, bb[uu]))))

    # tail flush
    for due, fn in sorted(deferred, key=lambda d: d[0]):
        fn()
    deferred = []

    dbg = io.get("dbg")
    if dbg is not None:
        nc.sync.dma_start(out=dbg["kt_a"], in_=KT8[0])
        nc.sync.dma_start(out=dbg["qt_a"], in_=QT8[0])
        nc.sync.dma_start(out=dbg["v"], in_=V)


def _build(debug_outputs=False):
    nc = bacc.Bacc("TRN2", debug=False, enable_asserts=False, num_devices=8)
    io = {
        "xb": nc.dram_tensor("xb", [NB, 128, 6, 512], BF16,
                             kind="ExternalInput").ap(),
        "w8": {n: nc.dram_tensor(n, [C, 128], BF16,
                                 kind="ExternalInput").ap()
               for n in ("wqk_a", "wqk_b")},
        "wvb": nc.dram_tensor("wvb", [C, 128], BF16, kind="ExternalInput").ap(),
        "wp": [nc.dram_tensor("wp_a", [D, C], F32R, kind="ExternalInput").ap(),
               nc.dram_tensor("wp_b", [D, C], F32R, kind="ExternalInput").ap()],
        "y": [nc.dram_tensor("y_a", [N, C], F32, kind="ExternalOutput").ap(),
              nc.dram_tensor("y_b", [N // 2, C], F32, kind="ExternalOutput").ap()],
        "rs": [nc.dram_tensor("rs_a", [NB, 512], F32R, kind="ExternalOutput").ap(),
               nc.dram_tensor("rs_b", [4, 512], F32R, kind="ExternalOutput").ap()],
    }
    if debug_outputs:
        io["dbg"] = {
            "kt_a": nc.dram_tensor("dbg_kt_a", [32, 2, N], FP8,
                                   kind="ExternalOutput").ap(),
            "qt_a": nc.dram_tensor("dbg_qt_a", [32, 2, N], FP8,
                                   kind="ExternalOutput").ap(),
            "v": nc.dram_tensor("dbg_v", [128, 16, 2, 2, 65], FP8,
                                kind="ExternalOutput").ap(),
        }
    from contextlib import ExitStack
    with tile.TileContext(nc) as tc, ExitStack() as ctx:
        _emit(nc, tc, io, ctx)
    nc.compile()
    return nc


def _get_nc():
    global _NC
    if _NC is None:
        _NC = _build()
    return _NC


def _in_maps(x, W_qkv, W_proj):
    xt_base = np.ascontiguousarray(x[0].T.astype(np.float32))  # [768, 4096]
    rot = np.concatenate([np.arange(2048, 4096), np.arange(0, 2048)])
    fp8 = ml_dtypes.float8_e4m3
    bf16 = ml_dtypes.bfloat16

    def wq(h):
        return W_qkv[h * 64:(h + 1) * 64, :]

    def wk(h):
        return W_qkv[C + h * 64:C + (h + 1) * 64, :]

    def wv(h):
        return W_qkv[2 * C + h * 64:2 * C + (h + 1) * 64, :]

    def qkb(h):  # K,Q of head h -> [768, 128] bf16, scaled by WS
        return np.ascontiguousarray(
            np.concatenate([wk(h).T, wq(h).T], axis=1) * WS).astype(bf16)

    maps = []
    for c in range(8):
        k = c // 2
        if c % 2 == 0:
            hA, hB = 3 * k, 3 * k + 1
            xt = xt_base
        else:
            hA, hB = 3 * k + 2, 3 * k + 1
            xt = np.ascontiguousarray(xt_base[:, rot])
        # per-nb contiguous slabs: xb[nb, p, cc, :] = xt[cc*128+p, nb*512+...]
        xb = np.ascontiguousarray(
            xt.reshape(6, 128, NB, 512).transpose(2, 1, 0, 3))
        maps.append({
            "xb": xb.astype(bf16),
            "wqk_a": qkb(hA), "wqk_b": qkb(hB),
            "wvb": np.ascontiguousarray(
                np.concatenate([wv(hA).T, wv(hB).T], axis=1)).astype(bf16),
            "wp_a": np.ascontiguousarray(W_proj[:, hA * 64:(hA + 1) * 64].T),
            "wp_b": np.ascontiguousarray(W_proj[:, hB * 64:(hB + 1) * 64].T),
        })
    return maps


def kernel(x, xpos, W_qkv, W_proj, b_proj, _results_hook=None):
    x = np.asarray(x, dtype=np.float32)
    W_qkv = np.asarray(W_qkv, dtype=np.float32)
    W_proj = np.asarray(W_proj, dtype=np.float32)
    b_proj = np.asarray(b_proj, dtype=np.float32)

    nc = _get_nc()
    res = run_bass_kernel_spmd(nc, _in_maps(x, W_qkv, W_proj),
                               core_ids=list(range(8)))
    if _results_hook is not None:
        _results_hook(res)

    rot = np.concatenate([np.arange(2048, 4096), np.arange(0, 2048)])
    out = np.zeros((N, C), np.float32)
    for c in range(8):
        r = res.results[c]
        gl = np.arange(N) if c % 2 == 0 else rot
        out[gl] += r["y_a"] / r["rs_a"].reshape(N)[:, None]
        out[gl[:2048]] += r["y_b"] / r["rs_b"].reshape(N // 2)[:, None]
    out += b_proj[None, :]
    return out[None]


# revision 42
# speedup vs baseline: 1.0156x; 1.0156x over previous
"""Trainium2 Bass kernel for multi-head attention (B=1, N=4096, C=768, H=12, D=64).

Sharding: tensor-parallel over heads across 8 cores. Core c (pair k=c//2):
  even c: head A = 3k   (all 8 query blocks), head B = 3k+1 (query blocks 0-3)
  odd  c: head A = 3k+2 (all 8 query blocks), head B = 3k+1 (query blocks 4-7)
The SPMD program is identical on every core; odd cores receive x^T with its
columns rotated by 2048 so that "local query blocks 0-3" of head B are the
global blocks 4-7.  The host un-permutes rows, normalizes by the softmax row
sums (computed on device via a ones-column appended to V), sums the per-core
partial projections and adds the bias.

v3 (cost-model driven):
  Q/K projection + scores + AV run as fp8e4 DoubleRow matmuls (2 k-subtiles
  per pass, 0.5 cyc/row).  Q^T/K^T live as [32, 2, N] fp8 tiles (d split in
  two 32-row k-tiles, W_qkv scaled x32; the x32*x32 factor is folded into the
  exp scale).  V is computed from bf16 x / bf16 W_v (fp8 V noise would pass
  straight to the output) and stored fp8 [128, 16, 2, 2, 65] with a ones
  column at dv=64 that yields softmax row sums in acc row 64.
  Scores S^T [m, q] accumulate in PSUM [128, 3, 512] tiles (3 key blocks per
  exp call); exp is split between ScalarE (exact, exp -> fp8) and VectorE
  (fast-exp: affine -> int8 -> bitcast fp8) with greedy load balancing,
  writing into a monolithic per-unit P^T tile [128, 32, 512] fp8 so AV can
  consume mb PAIRS via DoubleRow regardless of the exp grouping.
  Emission runs on an explicit step scheduler: scores(s) | AVs enabled by
  step s-1's exp | staged finalize (osb copy, then proj+y a step later);
  QKV projection blocks are interleaved right before the first score group
  that needs them.  Input DMAs split across the SP queue (fp8 x, qk weights)
  and the Pool/gpsimd queue (bf16 x, V/proj weights, all outputs).
"""

import sys

for _p in ("/opt/trn_rl_repo",):
    if _p not in sys.path:
        sys.path.insert(0, _p)

import ml_dtypes
import numpy as np

import concourse.bass as bass  # noqa: F401
import concourse.mybir as mybir
from concourse import bacc, tile
from concourse.bass_utils import run_bass_kernel_spmd

F32 = mybir.dt.float32
F32R = mybir.dt.float32r
BF16 = mybir.dt.bfloat16
FP8 = mybir.dt.float8e4
I8 = mybir.dt.int8
AF = mybir.ActivationFunctionType
DR = mybir.MatmulPerfMode.DoubleRow
MUL = mybir.AluOpType.mult
ADD = mybir.AluOpType.add
SUB = mybir.AluOpType.subtract

N = 4096
C = 768
D = 64
NB = 8  # 512-query/key blocks
WS = 32.0  # Q/K weight pre-scale (folded back out inside exp)
EXP_SCALE = (D ** -0.5) / (WS * WS)  # = 1/8192
# DVE fast-exp: i8 = round(raw * ALPHA + BETA); bitcast i8 -> fp8e4 ~ exp(raw*EXP_SCALE)
ALPHA = 8.0 * np.log2(np.e) * EXP_SCALE  # 1.4427/1024
BETA = 7 * 8 - 0.45  # fp8e4 bias 7 << 3 mantissa bits, Schraudolph-style centering

# groups of 2 key-blocks per exp call (PSUM: 2-bank score tiles x3 bufs)
GROUPS = [(2 * g, 2 * g + 2) for g in range(16)]
NG = len(GROUPS)
GW = 2  # max key-blocks per group

# per-free-element engine cost (ns) + fixed per-instruction cost, for greedy balance
ACT_CYC, ACT_FIX = 1.0 / 1.2, 242.0
DVE_CYC, DVE_FIX = 1.0 / 0.96, 230.0

_NC = None


def _emit(nc, tc, io, ctx):
    xb_in, w8_in, wvb_in, wp_in, y_out, rs_out = (
        io["xb"], io["w8"], io["wvb"], io["wp"], io["y"], io["rs"])

    sing = ctx.enter_context(tc.tile_pool(name="sing", bufs=1))
    spsum = ctx.enter_context(tc.tile_pool(name="spsum", bufs=3, space="PSUM"))
    apsum = ctx.enter_context(tc.tile_pool(name="apsum", bufs=2, space="PSUM"))
    ptp = ctx.enter_context(tc.tile_pool(name="ptp", bufs=3))
    osbp = ctx.enter_context(tc.tile_pool(name="osbp", bufs=3))
    ysbp = ctx.enter_context(tc.tile_pool(name="ysbp", bufs=6))

    # ---- greedy ACT/DVE load balancing for elementwise PSUM->SBUF work ----
    ew_load = {"act": 0.0, "dve": 0.0}

    def ew_pick(free):
        ca = ew_load["act"] + free * ACT_CYC + ACT_FIX
        cd = ew_load["dve"] + free * DVE_CYC + DVE_FIX
        if ca <= cd:
            ew_load["act"] = ca
            return "act"
        ew_load["dve"] = cd
        return "dve"

    def ew_copy(dst, src, free):
        if ew_pick(free) == "act":
            nc.scalar.copy(dst, src)
        else:
            nc.vector.tensor_copy(dst, src)

    def ew_exp(pt, ps, free):
        import os
        if os.environ.get("ALL_ACT_EXP") or ew_pick(free) == "act":
            nc.scalar.activation(out=pt, in_=ps, func=AF.Exp, scale=EXP_SCALE)
        else:
            nc.vector.tensor_scalar(pt.bitcast(I8), ps, ALPHA, BETA, MUL, ADD)

    # ---- weights: qk packed [K*32 | Q*32] bf16 (scores need bf16-grade
    # projection compute; only the Q/K stores are fp8 for the DR contract) ----
    wqk = {0: sing.tile([128, 6, 128], BF16, name="wqk_a", tag="wqk_a"),
           1: sing.tile([128, 6, 128], BF16, name="wqk_b", tag="wqk_b")}
    nc.sync.dma_start(
        out=wqk[0], in_=w8_in["wqk_a"].rearrange("(cc p) d -> p cc d", p=128))
    wvb = sing.tile([128, 6, 128], BF16, name="wvb", tag="wvb")
    wp = {0: sing.tile([64, C], F32R, name="wp_a", tag="wp_a"),
          1: sing.tile([64, C], F32R, name="wp_b", tag="wp_b")}

    # ---- projection result tiles ----
    KT8 = [sing.tile([32, 2, N], FP8, name="kt_a", tag="kt_a"),
           sing.tile([32, 2, N], FP8, name="kt_b", tag="kt_b")]
    QT8 = [sing.tile([32, 2, N], FP8, name="qt_a", tag="qt_a"),
           sing.tile([32, 2, N // 2], FP8, name="qt_b", tag="qt_b")]
    # last dim padded 65->80 so the AV DoubleRow k-tile stride (160) is 16-aligned.
    # V is stored as fp8 value + fp8 residual (V ~ V8 + R8) because V-element
    # quantization noise passes straight through to the output.
    V = sing.tile([128, 16, 2, 2, 80], FP8, name="v", tag="v")
    nc.vector.memset(V[:, :, :, :, 64:65], 1.0)
    R = sing.tile([128, 16, 2, 2, 80], FP8, name="vr", tag="vr")
    nc.vector.memset(R[:, :, :, :, 64:65], 0.0)

    # ---- x tiles: host pre-arranged per-nb contiguous slabs, nb-major so
    # the qkv pipeline can start early ----
    xbt = [sing.tile([128, 6, 512], BF16, name=f"xb_{nb}", tag=f"xb_{nb}")
           for nb in range(NB)]
    for nb in range(NB):
        nc.sync.dma_start(out=xbt[nb], in_=xb_in[nb])
        if nb == 0:
            nc.sync.dma_start(
                out=wqk[1],
                in_=w8_in["wqk_b"].rearrange("(cc p) d -> p cc d", p=128))
            nc.sync.dma_start(
                out=wvb, in_=wvb_in.rearrange("(cc p) d -> p cc d", p=128))
        elif nb == 1:
            nc.sync.dma_start(out=wp[0], in_=wp_in[0])
            nc.sync.dma_start(out=wp[1], in_=wp_in[1])

    # ---- QKV projections for one 512-column block ----
    # bf16 matmuls compute K AND Q for a head: psum partitions 0:64 K^T d,
    # 64:128 Q^T d; partition-shifted fp8 copies split it into the
    # [32, 2, N] DoubleRow score layout.
    def emit_qk_proj(s, nbp):  # nb pair (2*nbp, 2*nbp+1) in one psum tile
        pq = spsum.tile([128, 2, 512], F32, name="pq", tag="big")
        for h in range(2):
            for cc in range(6):
                nc.tensor.matmul(pq[:, h, :], lhsT=wqk[s][:, cc, :],
                                 rhs=xbt[2 * nbp + h][:, cc, :],
                                 start=(cc == 0), stop=(cc == 5))
        sl = slice(nbp * 1024, (nbp + 1) * 1024)
        ew_copy(KT8[s][:, 0, sl], pq[0:32], 1024)
        ew_copy(KT8[s][:, 1, sl], pq[32:64], 1024)
        if s == 0 or nbp < 2:
            ew_copy(QT8[s][:, 0, sl], pq[64:96], 1024)
            ew_copy(QT8[s][:, 1, sl], pq[96:128], 1024)

    def emit_v_proj(nb):
        # all 4 key-blocks of nb in one 1-bank psum tile (sub-bank matmul outs)
        psv = spsum.tile([128, 2, 2, 2, 64], F32, name="psv", tag="big")
        for idx in range(4):
            for cc in range(6):
                nc.tensor.matmul(psv[:, idx // 2, idx % 2, :, :],
                                 lhsT=xbt[nb][:, cc, idx * 128:(idx + 1) * 128],
                                 rhs=wvb[:, cc, :],
                                 start=(cc == 0), stop=(cc == 5))
        g0 = nb * 2
        dst_v = V[:, g0:g0 + 2, :, :, 0:64]
        ew_copy(dst_v, psv, 512)
        # fp8 residual on DVE: R8 = psv - float(V8)
        ew_load["dve"] += 512 * DVE_CYC + DVE_FIX
        nc.vector.scalar_tensor_tensor(R[:, g0:g0 + 2, :, :, 0:64], psv, 1.0,
                                       dst_v, MUL, SUB)

    # qkv work as a queue of fine chunks; the scheduler pops at most one
    # chunk per step (plus forced pops when a step's scores need the data)
    qkv_chunks = []
    for b in range(NB):
        if b % 2 == 0:
            qkv_chunks.append(("qk", 0, b // 2))
            qkv_chunks.append(("qk", 1, b // 2))
        qkv_chunks.append(("v", b, None))
    qkv_done_nb = -1
    qkv_pos = 0

    def pop_qkv():
        nonlocal qkv_pos, qkv_done_nb
        if qkv_pos >= len(qkv_chunks):
            return False
        kind, a, b = qkv_chunks[qkv_pos]
        if kind == "qk":
            emit_qk_proj(a, b)
        else:
            emit_v_proj(a)
            qkv_done_nb = a
        qkv_pos += 1
        return True

    def need_nb(nb):
        while qkv_done_nb < nb:
            pop_qkv()

    # ---- attention: 6 pair-slots, units = (slot, local qb) ----
    pairs = [((0, 0), (1, 0)), ((0, 1), (1, 1)), ((0, 2), (1, 2)),
             ((0, 3), (1, 3)), ((0, 4), (0, 5)), ((0, 6), (0, 7))]

    acc = {}
    for ulo, uup in pairs:
        for u in (ulo, uup):
            acc[u] = apsum.tile([65, 512], F32, name=f"acc_{u[0]}_{u[1]}",
                                tag="acc")

    def av_ready(g):
        # mb pairs fully covered by exp groups 0..g
        return min(16, (GROUPS[g][1]) // 2) if g >= 0 else 0

    def emit_av(pair, pt, k0, k1):
        for k in range(k0, k1):
            for u in pair:
                s, qb = u
                rhs = pt[u][:, 2 * k:2 * k + 2, :]
                nc.tensor.matmul(acc[u], lhsT=V[:, k, :, s, 0:65], rhs=rhs,
                                 start=(k == 0), stop=False,
                                 perf_mode=DR, skip_group_check=True)
                nc.tensor.matmul(acc[u], lhsT=R[:, k, :, s, 0:65], rhs=rhs,
                                 start=False, stop=(k == 15),
                                 perf_mode=DR, skip_group_check=True)

    def emit_osb(pair, osb_box):
        for u in pair:
            s, qb = u
            o = osbp.tile([65, 512], F32R, name="osb", tag="osb")
            ew_copy(o, acc[u], 512)
            nc.sync.dma_start(out=rs_out[s][qb:qb + 1, :], in_=o[64:65, :])
            osb_box[u] = o

    def emit_proj(u, osb):
        s, qb = u
        for qs in range(4):
            py = spsum.tile([128, C], F32, name="py", tag="big")
            lw = osb[0:64, qs * 128:(qs + 1) * 128]
            nc.tensor.matmul(py[:, 0:512], lhsT=lw, rhs=wp[s][:, 0:512],
                             start=True, stop=True)
            nc.tensor.matmul(py[:, 512:C], lhsT=lw, rhs=wp[s][:, 512:C],
                             start=True, stop=True)
            ysb = ysbp.tile([128, C], F32, name="ysb", tag="ysb")
            ew_copy(ysb, py, C)
            row = qb * 512 + qs * 128
            nc.sync.dma_start(out=y_out[s][row:row + 128, :], in_=ysb)

    # ---- explicit step scheduler ----
    # step s: scores+exp for (pair_s, g_s); AVs enabled by step s-2's exp
    # (two-step lag hides exp latency from the in-order PE stream); after a
    # pair's last AV batch: osb copies one step later, then each unit's
    # projection staged on the following steps.
    steps = [(pair, g) for pair in pairs for g in range(NG)]
    pt_tiles = {}
    av_done = {}
    deferred = []  # (due_step, fn)

    def flush_deferred(s):
        nonlocal deferred
        deferred, due = ([d for d in deferred if d[0] > s],
                         [d for d in deferred if d[0] <= s])
        for _, fn in sorted(due, key=lambda d: d[0]):
            fn()

    for s, (pair, g) in enumerate(steps):
        m0, m1 = GROUPS[g]
        need_nb(max(pair[0][1], pair[1][1], (m1 - 1) // 4))
        if g == 0:
            pt_tiles[pair] = {
                u: ptp.tile([128, 32, 512], FP8, name=f"pt_{u[0]}_{u[1]}",
                            tag="pt")
                for u in pair}
            av_done[pair] = 0
        # scores for this group
        ps = {}
        for u in pair:
            ps[u] = spsum.tile([128, GW, 512], F32, name="ps_s", tag="big")
            sl, qb = u
            for j in range(m1 - m0):
                mb = m0 + j
                nc.tensor.matmul(
                    ps[u][:, j, :],
                    lhsT=KT8[sl][:, :, mb * 128:(mb + 1) * 128],
                    rhs=QT8[sl][:, :, qb * 512:(qb + 1) * 512],
                    start=True, stop=True, perf_mode=DR)
        # at most one qkv chunk per step keeps PE bursts small
        if s > 0:
            pop_qkv()
        # deferred AV batches / finalize stages due at this step
        flush_deferred(s)
        # exp for this group
        for u in pair:
            w = (m1 - m0) * 512
            ew_exp(pt_tiles[pair][u][:, m0:m1, :], ps[u][:, 0:m1 - m0, :], w)
        # schedule this group's AVs two steps out (one for the last pair)
        last_pair = pair is pairs[-1]
        lag = 1 if (last_pair and g >= NG - 3) else 2
        k1 = av_ready(g)

        def av_batch(p=pair, a0=av_done[pair], a1=k1):
            emit_av(p, pt_tiles[p], a0, a1)
        deferred.append((s + lag, av_batch))
        av_done[pair] = k1
        if g == NG - 1:
            box = {}

            def osb_batch(p=pair, bb=box):
                emit_osb(p, bb)
            deferred.append((s + lag + 1, osb_batch))
            for i, u in enumerate(pair):
                deferred.append((s + lag + 2 + i, (lambda uu=u, bb=box:
                                                   emit_proj(uu, bb[uu]))))

    # tail flush
    for due, fn in sorted(deferred, key=lambda d: d[0]):
        fn()
    deferred = []

    dbg = io.get("dbg")
    if dbg is not None:
        nc.sync.dma_start(out=dbg["kt_a"], in_=KT8[0])
        nc.sync.dma_start(out=dbg["qt_a"], in_=QT8[0])
        nc.sync.dma_start(out=dbg["v"], in_=V)


def _build(debug_outputs=False):
    nc = bacc.Bacc("TRN2", debug=False, enable_asserts=False, num_devices=8)
    io = {
        "xb": nc.dram_tensor("xb", [NB, 128, 6, 512], BF16,
                             kind="ExternalInput").ap(),
        "w8": {n: nc.dram_tensor(n, [C, 128], BF16,
                                 kind="ExternalInput").ap()
               for n in ("wqk_a", "wqk_b")},
        "wvb": nc.dram_tensor("wvb", [C, 128], BF16, kind="ExternalInput").ap(),
        "wp": [nc.dram_tensor("wp_a", [D, C], F32R, kind="ExternalInput").ap(),
               nc.dram_tensor("wp_b", [D, C], F32R, kind="ExternalInput").ap()],
        "y": [nc.dram_tensor("y_a", [N, C], F32, kind="ExternalOutput").ap(),
              nc.dram_tensor("y_b", [N // 2, C], F32, kind="ExternalOutput").ap()],
        "rs": [nc.dram_tensor("rs_a", [NB, 512], F32R, kind="ExternalOutput").ap(),
               nc.dram_tensor("rs_b", [4, 512], F32R, kind="ExternalOutput").ap()],
    }
    if debug_outputs:
        io["dbg"] = {
            "kt_a": nc.dram_tensor("dbg_kt_a", [32, 2, N], FP8,
                                   kind="ExternalOutput").ap(),
            "qt_a": nc.dram_tensor("dbg_qt_a", [32, 2, N], FP8,
                                   kind="ExternalOutput").ap(),
            "v": nc.dram_tensor("dbg_v", [128, 16, 2, 2, 65], FP8,
                                kind="ExternalOutput").ap(),
        }
    from contextlib import ExitStack
    with tile.TileContext(nc) as tc, ExitStack() as ctx:
        _emit(nc, tc, io, ctx)
    nc.compile()
    return nc


def _get_nc():
    global _NC
    if _NC is None:
        _NC = _build()
    return _NC


def _in_maps(x, W_qkv, W_proj):
    xt_base = np.ascontiguousarray(x[0].T.astype(np.float32))  # [768, 4096]
    rot = np.concatenate([np.arange(2048, 4096), np.arange(0, 2048)])
    fp8 = ml_dtypes.float8_e4m3
    bf16 = ml_dtypes.bfloat16

    def wq(h):
        return W_qkv[h * 64:(h + 1) * 64, :]

    def wk(h):
        return W_qkv[C + h * 64:C + (h + 1) * 64, :]

    def wv(h):
        return W_qkv[2 * C + h * 64:2 * C + (h + 1) * 64, :]

    def qkb(h):  # K,Q of head h -> [768, 128] bf16, scaled by WS
        return np.ascontiguousarray(
            np.concatenate([wk(h).T, wq(h).T], axis=1) * WS).astype(bf16)

    maps = []
    for c in range(8):
        k = c // 2
        if c % 2 == 0:
            hA, hB = 3 * k, 3 * k + 1
            xt = xt_base
        else:
            hA, hB = 3 * k + 2, 3 * k + 1
            xt = np.ascontiguousarray(xt_base[:, rot])
        # per-nb contiguous slabs: xb[nb, p, cc, :] = xt[cc*128+p, nb*512+...]
        xb = np.ascontiguousarray(
            xt.reshape(6, 128, NB, 512).transpose(2, 1, 0, 3))
        maps.append({
            "xb": xb.astype(bf16),
            "wqk_a": qkb(hA), "wqk_b": qkb(hB),
            "wvb": np.ascontiguousarray(
                np.concatenate([wv(hA).T, wv(hB).T], axis=1)).astype(bf16),
            "wp_a": np.ascontiguousarray(W_proj[:, hA * 64:(hA + 1) * 64].T),
            "wp_b": np.ascontiguousarray(W_proj[:, hB * 64:(hB + 1) * 64].T),
        })
    return maps


def kernel(x, xpos, W_qkv, W_proj, b_proj, _results_hook=None):
    x = np.asarray(x, dtype=np.float32)
    W_qkv = np.asarray(W_qkv, dtype=np.float32)
    W_proj = np.asarray(W_proj, dtype=np.float32)
    b_proj = np.asarray(b_proj, dtype=np.float32)

    nc = _get_nc()
    res = run_bass_kernel_spmd(nc, _in_maps(x, W_qkv, W_proj),
                               core_ids=list(range(8)))
    if _results_hook is not None:
        _results_hook(res)

    rot = np.concatenate([np.arange(2048, 4096), np.arange(0, 2048)])
    out = np.zeros((N, C), np.float32)
    for c in range(8):
        r = res.results[c]
        gl = np.arange(N) if c % 2 == 0 else rot
        out[gl] += r["y_a"] / r["rs_a"].reshape(N)[:, None]
        out[gl[:2048]] += r["y_b"] / r["rs_b"].reshape(N // 2)[:, None]
    out += b_proj[None, :]
    return out[None]


# revision 59
# speedup vs baseline: 1.0710x; 1.0545x over previous
"""Trainium2 Bass kernel for multi-head attention (B=1, N=4096, C=768, H=12, D=64).

Sharding: tensor-parallel over heads across 8 cores. Core c (pair k=c//2):
  even c: head A = 3k   (all 8 query blocks), head B = 3k+1 (query blocks 0-3)
  odd  c: head A = 3k+2 (all 8 query blocks), head B = 3k+1 (query blocks 4-7)
The SPMD program is identical on every core; odd cores receive x^T with its
columns rotated by 2048 so that "local query blocks 0-3" of head B are the
global blocks 4-7.  The host un-permutes rows, normalizes by the softmax row
sums (computed on device via a ones-column appended to V), sums the per-core
partial projections and adds the bias.

Design (error budget measured against the 2e-2 gate; the attention signal is
~4x smaller than the bias so per-weight noise maps ~1:1 onto it):
  * Q/K projection runs in bf16 (fp8 projection inputs alone cost 1.7e-2 of
    output error); one matmul set per head computes K and Q together into a
    [128, 512] psum ([K-d | Q-d] columns), split by partition-shifted fp8
    copies into [32, 2, N] tiles (d in two 32-row k-subtiles, weights
    pre-scaled x32, folded back inside the exp scale).
  * Scores contract via fp8e4 DoubleRow (0.5 cyc/row, both d-subtiles per
    pass); S^T [m, q] accumulates in PSUM [128, 2, 512] tiles.
  * exp splits between ScalarE (exact exp -> fp8) and VectorE (fast-exp:
    affine -> int8 RNE -> bitcast fp8e4) with greedy load balancing, writing
    a monolithic per-unit P^T tile [128, 32, 512] fp8.
  * AV contracts mb PAIRS via DoubleRow with V stored fp8 (V-element noise
    passes straight to the output and costs 7e-3; acceptable inside the
    budget).  V's last dim is padded 65->80 so the DoubleRow ldweights
    k-tile stride (160) is 16-aligned, and a ones column at dv=64 yields
    softmax row sums in acc row 64.
  * DoubleRow ldweights requires all 128 PE columns (col_grp==0xf), which is
    why the packed projections and AV lhsT shapes look the way they do.
  * Emission runs on an explicit step scheduler: scores(s) | one qkv chunk |
    AVs from step s-2's exp | staged finalize (osb copy, per-unit projection
    on later steps).  QKV chunks are emitted just-in-time before the first
    score group that needs them.  x arrives as host-pre-arranged per-nb
    contiguous bf16 slabs so each DMA is 128 large descriptors.
"""

import sys

for _p in ("/opt/trn_rl_repo",):
    if _p not in sys.path:
        sys.path.insert(0, _p)

import ml_dtypes
import numpy as np

import concourse.bass as bass  # noqa: F401
import concourse.mybir as mybir
from concourse import bacc, tile
from concourse.bass_utils import run_bass_kernel_spmd

F32 = mybir.dt.float32
F32R = mybir.dt.float32r
BF16 = mybir.dt.bfloat16
FP8 = mybir.dt.float8e4
I8 = mybir.dt.int8
AF = mybir.ActivationFunctionType
DR = mybir.MatmulPerfMode.DoubleRow
MUL = mybir.AluOpType.mult
ADD = mybir.AluOpType.add
SUB = mybir.AluOpType.subtract

N = 4096
C = 768
D = 64
NB = 8  # 512-query/key blocks
WS = 32.0  # Q/K weight pre-scale (folded back out inside exp)
EXP_SCALE = (D ** -0.5) / (WS * WS)  # = 1/8192
# DVE fast-exp: i8 = round(raw * ALPHA + BETA); bitcast i8 -> fp8e4 ~ exp(raw*EXP_SCALE)
ALPHA = 8.0 * np.log2(np.e) * EXP_SCALE  # 1.4427/1024
BETA = 7 * 8 - 0.45  # fp8e4 bias 7 << 3 mantissa bits, Schraudolph-style centering

# groups of 2 key-blocks per exp call (PSUM: 2-bank score tiles x3 bufs)
GROUPS = [(2 * g, 2 * g + 2) for g in range(16)]
NG = len(GROUPS)
GW = 2  # max key-blocks per group

# per-free-element engine cost (ns) + fixed per-instruction cost, for greedy balance
ACT_CYC, ACT_FIX = 1.0 / 1.2, 242.0
DVE_CYC, DVE_FIX = 1.0 / 0.96, 230.0

_NC = None


def _emit(nc, tc, io, ctx):
    xb_in, w8_in, wvb_in, wp_in, y_out, rs_out = (
        io["xb"], io["w8"], io["wvb"], io["wp"], io["y"], io["rs"])

    sing = ctx.enter_context(tc.tile_pool(name="sing", bufs=1))
    apsum = ctx.enter_context(tc.tile_pool(name="apsum", bufs=2, space="PSUM"))
    ptp = ctx.enter_context(tc.tile_pool(name="ptp", bufs=4))
    spsum = ctx.enter_context(tc.tile_pool(name="spsum", bufs=3, space="PSUM"))
    osbp = ctx.enter_context(tc.tile_pool(name="osbp", bufs=6))
    ysbp = ctx.enter_context(tc.tile_pool(name="ysbp", bufs=12))

    # ---- greedy ACT/DVE load balancing for elementwise PSUM->SBUF work ----
    ew_load = {"act": 0.0, "dve": 0.0}

    def ew_pick(free):
        ca = ew_load["act"] + free * ACT_CYC + ACT_FIX
        cd = ew_load["dve"] + free * DVE_CYC + DVE_FIX
        if ca <= cd:
            ew_load["act"] = ca
            return "act"
        ew_load["dve"] = cd
        return "dve"

    def ew_copy(dst, src, free):
        if ew_pick(free) == "act":
            nc.scalar.copy(dst, src)
        else:
            nc.vector.tensor_copy(dst, src)

    def ew_exp(pt, ps, free):
        import os
        if os.environ.get("ALL_ACT_EXP") or ew_pick(free) == "act":
            nc.scalar.activation(out=pt, in_=ps, func=AF.Exp, scale=EXP_SCALE)
        else:
            nc.vector.tensor_scalar(pt.bitcast(I8), ps, ALPHA, BETA, MUL, ADD)

    # ---- weights: qk packed [K*32 | Q*32] bf16 (scores need bf16-grade
    # projection compute; only the Q/K stores are fp8 for the DR contract) ----
    wqk = {0: sing.tile([128, 6, 128], BF16, name="wqk_a", tag="wqk_a"),
           1: sing.tile([128, 6, 128], BF16, name="wqk_b", tag="wqk_b")}
    nc.sync.dma_start(
        out=wqk[0], in_=w8_in["wqk_a"].rearrange("(cc p) d -> p cc d", p=128))
    wvb = sing.tile([128, 6, 128], BF16, name="wvb", tag="wvb")
    wp = {0: sing.tile([64, C], F32R, name="wp_a", tag="wp_a"),
          1: sing.tile([64, C], F32R, name="wp_b", tag="wp_b")}

    # warm the ScalarE exp table during the input-DMA window
    warm = sing.tile([1, 1], F32, name="warm", tag="warm")
    nc.vector.memset(warm, 0.0)
    warm8 = sing.tile([1, 1], FP8, name="warm8", tag="warm8")
    nc.scalar.activation(out=warm8, in_=warm, func=AF.Exp, scale=1.0)

    # ---- projection result tiles ----
    KT8 = [sing.tile([32, 2, N], FP8, name="kt_a", tag="kt_a"),
           sing.tile([32, 2, N], FP8, name="kt_b", tag="kt_b")]
    QT8 = [sing.tile([32, 2, N], FP8, name="qt_a", tag="qt_a"),
           sing.tile([32, 2, N // 2], FP8, name="qt_b", tag="qt_b")]
    # last dim padded 65->80 so the AV DoubleRow k-tile stride (160) is 16-aligned.
    # V is stored as fp8 value + fp8 residual (V ~ V8 + R8) because V-element
    # quantization noise passes straight through to the output.
    V = sing.tile([128, 16, 2, 2, 80], FP8, name="v", tag="v")
    nc.vector.memset(V[:, :, :, :, 64:65], 1.0)
    R = None
    if bool(_os.environ.get("VRES")):
        R = sing.tile([128, 16, 2, 2, 80], FP8, name="vr", tag="vr")
        nc.vector.memset(R[:, :, :, :, 64:65], 0.0)

    # ---- x tiles: host pre-arranged per-nb contiguous slabs, nb-major so
    # the qkv pipeline can start early ----
    xbt = [sing.tile([128, 6, 512], BF16, name=f"xb_{nb}", tag=f"xb_{nb}")
           for nb in range(NB)]
    for nb in range(NB):
        if nb == 0:
            nc.sync.dma_start(out=xbt[0][:, 0:3, :], in_=xb_in[0][:, 0:3, :])
            nc.sync.dma_start(out=xbt[0][:, 3:6, :], in_=xb_in[0][:, 3:6, :])
        else:
            nc.sync.dma_start(out=xbt[nb], in_=xb_in[nb])
        if nb == 0:
            nc.sync.dma_start(
                out=wqk[1],
                in_=w8_in["wqk_b"].rearrange("(cc p) d -> p cc d", p=128))
            nc.sync.dma_start(
                out=wvb, in_=wvb_in.rearrange("(cc p) d -> p cc d", p=128))
        elif nb == 1:
            nc.sync.dma_start(out=wp[0], in_=wp_in[0])
            nc.sync.dma_start(out=wp[1], in_=wp_in[1])

    # ---- QKV projections for one 512-column block ----
    # bf16 matmuls compute K AND Q for a head: psum partitions 0:64 K^T d,
    # 64:128 Q^T d; partition-shifted fp8 copies split it into the
    # [32, 2, N] DoubleRow score layout.
    def emit_qk_proj(s, nbp):  # nb pair (2*nbp, 2*nbp+1) in one psum tile
        pq = spsum.tile([128, 2, 512], F32, name="pq", tag="big")
        for h in range(2):
            for cc in range(6):
                nc.tensor.matmul(pq[:, h, :], lhsT=wqk[s][:, cc, :],
                                 rhs=xbt[2 * nbp + h][:, cc, :],
                                 start=(cc == 0), stop=(cc == 5))
        sl = slice(nbp * 1024, (nbp + 1) * 1024)
        ew_copy(KT8[s][:, 0, sl], pq[0:32], 1024)
        ew_copy(KT8[s][:, 1, sl], pq[32:64], 1024)
        if s == 0 or nbp < 2:
            ew_copy(QT8[s][:, 0, sl], pq[64:96], 1024)
            ew_copy(QT8[s][:, 1, sl], pq[96:128], 1024)

    def emit_v_proj(nb):
        # all 4 key-blocks of nb in one 1-bank psum tile (sub-bank matmul outs)
        psv = spsum.tile([128, 2, 2, 2, 64], F32, name="psv", tag="big")
        for idx in range(4):
            for cc in range(6):
                nc.tensor.matmul(psv[:, idx // 2, idx % 2, :, :],
                                 lhsT=xbt[nb][:, cc, idx * 128:(idx + 1) * 128],
                                 rhs=wvb[:, cc, :],
                                 start=(cc == 0), stop=(cc == 5))
        g0 = nb * 2
        dst_v = V[:, g0:g0 + 2, :, :, 0:64]
        ew_copy(dst_v, psv, 512)
        if bool(_os.environ.get("VRES")):
            # fp8 residual on DVE: R8 = psv - float(V8)
            ew_load["dve"] += 512 * DVE_CYC + DVE_FIX
            nc.vector.scalar_tensor_tensor(R[:, g0:g0 + 2, :, :, 0:64], psv,
                                           1.0, dst_v, MUL, SUB)

    # qkv work as a queue of fine chunks; the scheduler pops at most one
    # chunk per step (plus forced pops when a step's scores need the data)
    qkv_chunks = []
    for b in range(NB):
        if b % 2 == 0:
            qkv_chunks.append(("qk", 0, b // 2))
            qkv_chunks.append(("qk", 1, b // 2))
        qkv_chunks.append(("v", b))
    qkv_done_nb = -1
    qkv_pos = 0

    def pop_qkv():
        nonlocal qkv_pos, qkv_done_nb
        if qkv_pos >= len(qkv_chunks):
            return False
        chunk = qkv_chunks[qkv_pos]
        if chunk[0] == "qk":
            emit_qk_proj(chunk[1], chunk[2])
        else:
            emit_v_proj(chunk[1])
            qkv_done_nb = chunk[1]
        qkv_pos += 1
        return True

    def need_nb(nb):
        while qkv_done_nb < nb:
            pop_qkv()

    # ---- attention: 6 pair-slots, units = (slot, local qb) ----
    pairs = [((0, 0), (1, 0)), ((0, 1), (1, 1)), ((0, 2), (1, 2)),
             ((0, 3), (1, 3)), ((0, 4), (0, 5)), ((0, 6), (0, 7))]

    acc = {}
    for ulo, uup in pairs:
        for u in (ulo, uup):
            acc[u] = apsum.tile([65, 512], F32, name=f"acc_{u[0]}_{u[1]}",
                                tag="acc")

    def av_ready(g):
        # mb pairs fully covered by exp groups 0..g
        return min(16, (GROUPS[g][1]) // 2) if g >= 0 else 0

    VRES = bool(_os.environ.get("VRES"))

    def emit_av(pair, pt, k0, k1):
        for k in range(k0, k1):
            for u in pair:
                s, qb = u
                rhs = pt[u][:, 2 * k:2 * k + 2, :]
                nc.tensor.matmul(acc[u], lhsT=V[:, k, :, s, 0:65], rhs=rhs,
                                 start=(k == 0), stop=(not VRES and k == 15),
                                 perf_mode=DR, skip_group_check=True)
                if VRES:
                    nc.tensor.matmul(acc[u], lhsT=R[:, k, :, s, 0:65],
                                     rhs=rhs, start=False, stop=(k == 15),
                                     perf_mode=DR, skip_group_check=True)

    def emit_osb(pair, osb_box):
        for u in pair:
            s, qb = u
            o = osbp.tile([65, 512], F32R, name="osb", tag="osb")
            ew_copy(o, acc[u], 512)
            nc.sync.dma_start(out=rs_out[s][qb:qb + 1, :], in_=o[64:65, :])
            osb_box[u] = o

    def emit_proj(u, osb):
        s, qb = u
        for qs in range(4):
            py = spsum.tile([128, C], F32, name="py", tag="big")
            lw = osb[0:64, qs * 128:(qs + 1) * 128]
            nc.tensor.matmul(py[:, 0:512], lhsT=lw, rhs=wp[s][:, 0:512],
                             start=True, stop=True)
            nc.tensor.matmul(py[:, 512:C], lhsT=lw, rhs=wp[s][:, 512:C],
                             start=True, stop=True)
            ysb = ysbp.tile([128, C], F32, name="ysb", tag="ysb")
            ew_copy(ysb, py, C)
            row = qb * 512 + qs * 128
            nc.sync.dma_start(out=y_out[s][row:row + 128, :], in_=ysb)

    # ---- explicit step scheduler ----
    # step s: scores+exp for (pair_s, g_s); AVs enabled by step s-2's exp
    # (two-step lag hides exp latency from the in-order PE stream); after a
    # pair's last AV batch: osb copies one step later, then each unit's
    # projection staged on the following steps.
    steps = [(pair, g) for pair in pairs for g in range(NG)]
    pt_tiles = {}
    av_done = {}
    deferred = []  # (due_step, fn)

    def flush_deferred(s):
        nonlocal deferred
        deferred, due = ([d for d in deferred if d[0] > s],
                         [d for d in deferred if d[0] <= s])
        for _, fn in sorted(due, key=lambda d: d[0]):
            fn()

    for s, (pair, g) in enumerate(steps):
        m0, m1 = GROUPS[g]
        need_nb(max(pair[0][1], pair[1][1], (m1 - 1) // 4))
        if g == 0:
            pt_tiles[pair] = {
                u: ptp.tile([128, 32, 512], FP8, name=f"pt_{u[0]}_{u[1]}",
                            tag="pt")
                for u in pair}
            av_done[pair] = 0
        # scores for this group
        ps = {}
        for u in pair:
            ps[u] = spsum.tile([128, GW, 512], F32, name="ps_s", tag="big")
            sl, qb = u
            for j in range(m1 - m0):
                mb = m0 + j
                nc.tensor.matmul(
                    ps[u][:, j, :],
                    lhsT=KT8[sl][:, :, mb * 128:(mb + 1) * 128],
                    rhs=QT8[sl][:, :, qb * 512:(qb + 1) * 512],
                    start=True, stop=True, perf_mode=DR)
        # at most one qkv chunk per step keeps PE bursts small
        if s > 0:
            pop_qkv()
        # deferred AV batches / finalize stages due at this step
        flush_deferred(s)
        # exp for this group
        for u in pair:
            w = (m1 - m0) * 512
            ew_exp(pt_tiles[pair][u][:, m0:m1, :], ps[u][:, 0:m1 - m0, :], w)
        # schedule this group's AVs two steps out (one for the last pair)
        last_pair = pair is pairs[-1]
        base_lag = int(_os.environ.get("AV_LAG", 2))
        lag = 1 if (last_pair and g >= NG - 3) else base_lag
        k1 = av_ready(g)

        def av_batch(p=pair, a0=av_done[pair], a1=k1):
            emit_av(p, pt_tiles[p], a0, a1)
        deferred.append((s + lag, av_batch))
        av_done[pair] = k1
        if g == NG - 1:
            box = {}

            def osb_batch(p=pair, bb=box):
                emit_osb(p, bb)
            osb_off = int(_os.environ.get("OSB_OFF", 1))
            proj_off = int(_os.environ.get("PROJ_OFF", 1))
            deferred.append((s + lag + osb_off, osb_batch))
            for i, u in enumerate(pair):
                deferred.append((s + lag + proj_off + i, (lambda uu=u, bb=box:
                                                          emit_proj(uu, bb[uu]))))

    # tail flush
    for due, fn in sorted(deferred, key=lambda d: d[0]):
        fn()
    deferred = []

    dbg = io.get("dbg")
    if dbg is not None:
        nc.sync.dma_start(out=dbg["kt_a"], in_=KT8[0])
        nc.sync.dma_start(out=dbg["qt_a"], in_=QT8[0])
        nc.sync.dma_start(out=dbg["v"], in_=V)


def _build(debug_outputs=False):
    nc = bacc.Bacc("TRN2", debug=False, enable_asserts=False, num_devices=8)
    io = {
        "xb": nc.dram_tensor("xb", [NB, 128, 6, 512], BF16,
                             kind="ExternalInput").ap(),
        "w8": {n: nc.dram_tensor(n, [C, 128], BF16,
                                 kind="ExternalInput").ap()
               for n in ("wqk_a", "wqk_b")},
        "wvb": nc.dram_tensor("wvb", [C, 128], BF16, kind="ExternalInput").ap(),
        "wp": [nc.dram_tensor("wp_a", [D, C], F32R, kind="ExternalInput").ap(),
               nc.dram_tensor("wp_b", [D, C], F32R, kind="ExternalInput").ap()],
        "y": [nc.dram_tensor("y_a", [N, C], F32, kind="ExternalOutput").ap(),
              nc.dram_tensor("y_b", [N // 2, C], F32, kind="ExternalOutput").ap()],
        "rs": [nc.dram_tensor("rs_a", [NB, 512], F32R, kind="ExternalOutput").ap(),
               nc.dram_tensor("rs_b", [4, 512], F32R, kind="ExternalOutput").ap()],
    }
    if debug_outputs:
        io["dbg"] = {
            "kt_a": nc.dram_tensor("dbg_kt_a", [32, 2, N], FP8,
                                   kind="ExternalOutput").ap(),
            "qt_a": nc.dram_tensor("dbg_qt_a", [32, 2, N], FP8,
                                   kind="ExternalOutput").ap(),
            "v": nc.dram_tensor("dbg_v", [128, 16, 2, 2, 65], FP8,
                                kind="ExternalOutput").ap(),
        }
    from contextlib import ExitStack
    with tile.TileContext(nc) as tc, ExitStack() as ctx:
        _emit(nc, tc, io, ctx)
    nc.compile()
    return nc


def _get_nc():
    global _NC
    if _NC is None:
        _NC = _build()
    return _NC


def _in_maps(x, W_qkv, W_proj):
    xt_base = np.ascontiguousarray(x[0].T.astype(np.float32))  # [768, 4096]
    rot = np.concatenate([np.arange(2048, 4096), np.arange(0, 2048)])
    fp8 = ml_dtypes.float8_e4m3
    bf16 = ml_dtypes.bfloat16

    def wq(h):
        return W_qkv[h * 64:(h + 1) * 64, :]

    def wk(h):
        return W_qkv[C + h * 64:C + (h + 1) * 64, :]

    def wv(h):
        return W_qkv[2 * C + h * 64:2 * C + (h + 1) * 64, :]

    def qkb(h):  # K,Q of head h -> [768, 128] bf16, scaled by WS
        return np.ascontiguousarray(
            np.concatenate([wk(h).T, wq(h).T], axis=1) * WS).astype(bf16)

    maps = []
    for c in range(8):
        k = c // 2
        if c % 2 == 0:
            hA, hB = 3 * k, 3 * k + 1
            xt = xt_base
        else:
            hA, hB = 3 * k + 2, 3 * k + 1
            xt = np.ascontiguousarray(xt_base[:, rot])
        # per-nb contiguous slabs: xb[nb, p, cc, :] = xt[cc*128+p, nb*512+...]
        xb = np.ascontiguousarray(
            xt.reshape(6, 128, NB, 512).transpose(2, 1, 0, 3))
        maps.append({
            "xb": xb.astype(bf16),
            "wqk_a": qkb(hA), "wqk_b": qkb(hB),
            "wvb": np.ascontiguousarray(
                np.concatenate([wv(hA).T, wv(hB).T], axis=1)).astype(bf16),
            "wp_a": np.ascontiguousarray(W_proj[:, hA * 64:(hA + 1) * 64].T),
            "wp_b": np.ascontiguousarray(W_proj[:, hB * 64:(hB + 1) * 64].T),
        })
    return maps


def kernel(x, xpos, W_qkv, W_proj, b_proj, _results_hook=None):
    x = np.asarray(x, dtype=np.float32)
    W_qkv = np.asarray(W_qkv, dtype=np.float32)
    W_proj = np.asarray(W_proj, dtype=np.float32)
    b_proj = np.asarray(b_proj, dtype=np.float32)

    nc = _get_nc()
    res = run_bass_kernel_spmd(nc, _in_maps(x, W_qkv, W_proj),
                               core_ids=list(range(8)))
    if _results_hook is not None:
        _results_hook(res)

    rot = np.concatenate([np.arange(2048, 4096), np.arange(0, 2048)])
    out = np.zeros((N, C), np.float32)
    for c in range(8):
        r = res.results[c]
        gl = np.arange(N) if c % 2 == 0 else rot
        out[gl] += r["y_a"] / r["rs_a"].reshape(N)[:, None]
        out[gl[:2048]] += r["y_b"] / r["rs_b"].reshape(N // 2)[:, None]
    out += b_proj[None, :]
    return out[None]
